# revision 12
# baseline (speedup 1.0000x reference)
"""Multi-head attention (B=2, N=2048, D=768, H=12) on 8 Trainium2 NeuronCores.

Sharding: data-parallel over rows of (B*N) with redundant K/V projection.
Each core c handles batch b=c//4 and query rows q0=(c%4)*512 .. q0+512.
It receives the full batch's x (rolled so its own query rows come first --
softmax over keys is permutation invariant, so key order doesn't matter),
computes K/V for the whole batch (4x redundant) plus Q for its own rows,
runs attention + output projection for its rows, and returns [512, 768].
No cross-core communication is needed.

Device-side layout strategy:
  - x is transposed on the PE (via identity matmuls) into x^T [768, seq]
    d-major tiles, which serve as rhs (K^T/Q^T projections) and lhsT (V).
  - K^T, Q^T are produced directly in [d, seq] layout (fp32r) so the
    scores matmul needs no further transposes; scores come out as
    scores^T [keys, q] in PSUM, exp runs on ACT (scale folded in,
    no max-subtraction needed at these magnitudes), P stored bf16.
  - V tiles are stored bf16 as [V_h0 | 1 | V_h1 | 1] per head-pair so a
    single M=65 matmul per key-chunk yields attnout^T rows 0..63 and the
    softmax denominator (row 64) for free.
  - attnout^T is normalized per head (reciprocal + gpsimd partition
    broadcast) straight into the lhsT layout the output projection needs.
All matmuls run in bf16 (inputs pre-rounded host-side), accumulating in
fp32 PSUM. The softmax scale (1/8) keeps the bf16 score error ~3e-3 in the
exponent, so the end-to-end relative error stays in the few-1e-3 range.
"""

import sys

sys.path.insert(0, "/opt/trn_rl_repo")

import numpy as np

import concourse.bass as bass
import concourse.mybir as mybir
import concourse.tile as tile
from concourse import bacc
from concourse import bass_utils
from concourse.masks import make_identity

B, N, D = 2, 2048, 768
H, DH = 12, 64
NCORES = 8
S = 2048          # keys per batch
SQ = 512          # query rows per core
NSC = S // 512    # seq chunks (of 512) for the projections
NKC = S // 128    # key chunks (of 128) for attention
NPAIR = H // 2    # head pairs
KC = D // 128     # contraction chunks
SCALE = DH ** -0.5

f32 = mybir.dt.float32
f32r = mybir.dt.float32r
bf16 = mybir.dt.bfloat16
ADD = mybir.AluOpType.add
EXP = mybir.ActivationFunctionType.Exp

_CACHE = {}


def _build():
    nc = bacc.Bacc("TRN2", target_bir_lowering=False, debug=False,
                   enable_asserts=False, num_devices=NCORES)
    xb = nc.dram_tensor("xb", [S, D], bf16, kind="ExternalInput").ap()
    wqkv = nc.dram_tensor("wqkv", [D, 3 * D], bf16, kind="ExternalInput").ap()
    bqkv = nc.dram_tensor("bqkv", [3 * D], f32, kind="ExternalInput").ap()
    wproj = nc.dram_tensor("wproj", [D, D], bf16, kind="ExternalInput").ap()
    bproj = nc.dram_tensor("bproj", [D], f32, kind="ExternalInput").ap()
    out = nc.dram_tensor("out", [SQ, D], f32, kind="ExternalOutput").ap()

    with tile.TileContext(nc) as tc:
        from contextlib import ExitStack
        with ExitStack() as stack:
            ep = lambda *a, **k: stack.enter_context(tc.tile_pool(*a, **k))
            consts = ep(name="consts", bufs=1)
            wv_pool = ep(name="wv_pool", bufs=1)
            kt_pool = ep(name="kt_pool", bufs=1)
            qt_pool = ep(name="qt_pool", bufs=1)
            v_pool = ep(name="v_pool", bufs=1)
            at_pool = ep(name="at_pool", bufs=1)
            wq_pool = ep(name="wq_pool", bufs=1)
            wk_pool = ep(name="wk_pool", bufs=1)
            xn_pool = ep(name="xn_pool", bufs=8)
            xt_pool = ep(name="xt_pool", bufs=2)
            p_pool = ep(name="p_pool", bufs=3)
            nrm_pool = ep(name="nrm_pool", bufs=2)
            acc_pool = ep(name="acc_pool", bufs=1)
            wp_pool = ep(name="wp_pool", bufs=1)
            outp = ep(name="outp", bufs=2)
            ps1 = ep(name="ps1", bufs=2, space="PSUM")
            ps_av = ep(name="ps_av", bufs=2, space="PSUM")
            ps2 = ep(name="ps2", bufs=2, space="PSUM")

            # ---- constants (x rows for chunk 0 are queued first so the
            # transposes can start while the weight DMAs stream in) ----
            ident = consts.tile([128, 128], bf16)
            make_identity(nc, ident)
            # bqkv as [128, 18]: col j holds bqkv[128j .. 128j+127]
            bq_sb = consts.tile([128, 18], f32)
            nc.sync.dma_start(out=bq_sb, in_=bqkv.rearrange("(j p) -> p j", p=128))
            # bproj broadcast to all partitions
            bp_bc = consts.tile([128, D], f32)
            bp_in = bass.AP(tensor=bproj.tensor, offset=bproj.offset,
                            ap=[[0, 128]] + list(bproj.ap))
            nc.gpsimd.dma_start(out=bp_bc, in_=bp_in)

            xns_by_s = {}

            def load_xn(s):
                xns = []
                for jj in range(4):
                    xn = xn_pool.tile([128, D], bf16, name=f"xn{s}_{jj}", tag="xn")
                    nc.sync.dma_start(
                        out=xn,
                        in_=xb[s * 512 + jj * 128: s * 512 + (jj + 1) * 128, :])
                    xns.append(xn)
                xns_by_s[s] = xns

            load_xn(0)

            # ---- persistent operand tiles ----
            wq, wk, wv, wp = [], [], [], []
            for c in range(KC):
                rows = slice(c * 128, (c + 1) * 128)
                wkt = wk_pool.tile([128, D], bf16, name=f"wk{c}", tag=f"wk{c}")
                nc.sync.dma_start(out=wkt, in_=wqkv[rows, D:2 * D])
                wk.append(wkt)
            for c in range(KC):
                rows = slice(c * 128, (c + 1) * 128)
                wqt = wq_pool.tile([128, D], bf16, name=f"wq{c}", tag=f"wq{c}")
                nc.sync.dma_start(out=wqt, in_=wqkv[rows, 0:D])
                wq.append(wqt)
            for c in range(KC):
                rows = slice(c * 128, (c + 1) * 128)
                wvt = wv_pool.tile([128, D], bf16, name=f"wv{c}", tag=f"wv{c}")
                nc.sync.dma_start(out=wvt, in_=wqkv[rows, 2 * D:3 * D])
                wv.append(wvt)
            kt = [kt_pool.tile([128, S], bf16, name=f"kt{j}", tag=f"kt{j}")
                  for j in range(NPAIR)]
            qt = [qt_pool.tile([128, SQ], bf16, name=f"qt{j}", tag=f"qt{j}")
                  for j in range(NPAIR)]
            vt = [v_pool.tile([128, NPAIR * 130], bf16, name=f"vt{k}", tag=f"vt{k}")
                  for k in range(NKC)]
            at = [at_pool.tile([128, SQ], bf16, name=f"at{j}", tag=f"at{j}")
                  for j in range(NPAIR)]
            # SBUF fp32 accumulators for AV across 4-chunk windows (row 64 =
            # softmax denominator); they exist so only one pair of AV PSUM
            # tiles is live at a time, letting attention overlap phase A.
            acc = [[acc_pool.tile([65, 512], f32, name=f"acc{j}_{h}",
                                  tag=f"acc{j}_{h}") for h in range(2)]
                   for j in range(NPAIR)]

            # ones columns of the V tiles (col 64 and 129 of each pair block)
            for k in range(NKC):
                nc.vector.memset(
                    vt[k].rearrange("p (j t h) -> p j t h", j=NPAIR, t=2)[:, :, :, 64:65],
                    1.0)

            # ---- main wave ----
            # Tile's schedule is static per engine, so attention (ACT-paced,
            # PE half-idle) must be hand-interleaved with the next chunk's
            # projections or PE stalls at every exp gate. Each head pair of
            # attention(s) is followed by a piece of the chunk-(s+1)
            # projections in emission order.
            xts_by_s = {}

            def emit_transposes(s, cs):
                xns = xns_by_s[s]
                xts = xts_by_s.setdefault(s, {})
                for c in cs:
                    tp = ps1.tile([128, 512], bf16, name=f"tp{s}_{c}", tag="ps1")
                    for jj in range(4):
                        nc.tensor.transpose(
                            tp[:, jj * 128:(jj + 1) * 128],
                            xns[jj][:, c * 128:(c + 1) * 128], ident[:])
                    xtc = xt_pool.tile([128, 512], bf16, name=f"xt{s}_{c}",
                                       tag=f"xt{c}")
                    nc.vector.tensor_copy(xtc, tp)
                    xts[c] = xtc

            def emit_q(s):
                xts = xts_by_s[s]
                for j in range(NPAIR):
                    qp = ps1.tile([128, 512], f32, name=f"qp{j}", tag="ps1")
                    for c in range(KC):
                        nc.tensor.matmul(qp, wq[c][:, j * 128:(j + 1) * 128],
                                         xts[c][:],
                                         start=(c == 0), stop=(c == KC - 1))
                    nc.vector.tensor_scalar_add(qt[j], qp, bq_sb[:, j:j + 1])

            def emit_k(s, js):
                xts = xts_by_s[s]
                for j in js:
                    kp = ps1.tile([128, 512], f32, name=f"kp{j}_{s}", tag="ps1")
                    for c in range(KC):
                        nc.tensor.matmul(kp, wk[c][:, j * 128:(j + 1) * 128],
                                         xts[c][:],
                                         start=(c == 0), stop=(c == KC - 1))
                    nc.vector.tensor_scalar_add(
                        kt[j][:, s * 512:(s + 1) * 512], kp, bq_sb[:, 6 + j:7 + j])

            def emit_v(s, ms):
                xts = xts_by_s[s]
                for m in ms:
                    k = s * 4 + m
                    vp = ps2.tile([128, 1024], f32, name=f"vp{k}", tag="ps2")
                    for c in range(KC):
                        nc.tensor.matmul(vp[:, 0:512],
                                         xts[c][:, m * 128:(m + 1) * 128],
                                         wv[c][:, 0:512],
                                         start=(c == 0), stop=(c == KC - 1))
                    for c in range(KC):
                        nc.tensor.matmul(vp[:, 512:768],
                                         xts[c][:, m * 128:(m + 1) * 128],
                                         wv[c][:, 512:768],
                                         start=(c == 0), stop=(c == KC - 1))
                    nc.vector.tensor_copy(
                        vt[k].rearrange("p (j t h) -> p j t h",
                                        j=NPAIR, t=2)[:, :, :, 0:64],
                        vp[:, 0:768].rearrange("p (j t h) -> p j t h",
                                               j=NPAIR, t=2))

            def emit_attn_pair(j, s):
                # two heads as row-tiled concurrent matmuls (lhsT partition
                # bases 0/64 -> disjoint PE row strips, outputs in different
                # PSUM banks of one 2-bank tile); one ACTIVATE exps both
                # heads' scores per key chunk.
                av_e = ps_av.tile([128, 512], f32, name=f"ave{j}_{s}", tag="av")
                av_o = ps_av.tile([128, 512], f32, name=f"avo{j}_{s}", tag="av")
                for m in range(4):
                    k = s * 4 + m
                    sc = ps2.tile([128, 1024], f32, name=f"sc{j}_{k}", tag="ps2")
                    nc.tensor.matmul(sc[:, 0:512],
                                     kt[j][0:64, k * 128:(k + 1) * 128],
                                     qt[j][0:64, :], start=True, stop=True)
                    nc.tensor.matmul(sc[:, 512:1024],
                                     kt[j][64:128, k * 128:(k + 1) * 128],
                                     qt[j][64:128, :], start=True, stop=True)
                    pt = p_pool.tile([128, 1024], bf16, name=f"p{j}_{k}", tag="p")
                    nc.scalar.activation(pt, sc, EXP, scale=SCALE)
                    nc.tensor.matmul(av_e[0:65, :],
                                     vt[k][:, j * 130: j * 130 + 65],
                                     pt[:, 0:512],
                                     start=(m == 0), stop=(m == 3))
                    nc.tensor.matmul(av_o[0:65, :],
                                     vt[k][:, j * 130 + 65: j * 130 + 130],
                                     pt[:, 512:1024],
                                     start=(m == 0), stop=(m == 3))
                for h, av in ((0, av_e), (1, av_o)):
                    if s == 0:
                        nc.vector.tensor_copy(acc[j][h], av[0:65, :])
                    else:
                        nc.vector.tensor_tensor(acc[j][h], acc[j][h],
                                                av[0:65, :], ADD)

            # chunk-0 projections up front
            emit_transposes(0, range(KC))
            emit_q(0)
            emit_k(0, range(NPAIR))
            emit_v(0, range(4))
            # pieces of chunk s+1's projections, slotted between the six
            # attention pairs of chunk s (transposes first, then K, then V)
            pieces = [lambda s: emit_transposes(s, (0, 1, 2)),
                      lambda s: emit_transposes(s, (3, 4, 5)),
                      lambda s: emit_k(s, (0, 1, 2)),
                      lambda s: emit_k(s, (3, 4, 5)),
                      lambda s: emit_v(s, (0, 1)),
                      lambda s: emit_v(s, (2, 3))]

            for s in range(NSC):
                if s + 1 < NSC:
                    load_xn(s + 1)
                if s == 0:
                    # load W_proj behind the first attention wave
                    for c in range(KC):
                        wpt = wp_pool.tile([128, D], bf16, name=f"wp{c}",
                                           tag=f"wp{c}")
                        nc.sync.dma_start(out=wpt,
                                          in_=wproj[c * 128:(c + 1) * 128, :])
                        wp.append(wpt)
                for j in range(NPAIR):
                    emit_attn_pair(j, s)
                    if s + 1 < NSC:
                        pieces[j](s + 1)
                xns_by_s.pop(s, None)
                if s > 0:
                    xts_by_s.pop(s - 1, None)

            # ---- normalize into attnout^T (and add v-bias; exact since
            # (V+1 b_v)^T P / sums = V^T P / sums + b_v) ----
            for j in range(NPAIR):
                # Both heads' sums share one reciprocal op: rows 0 and 64
                # (the only partition bases engines accept); unused rows are
                # memset to 1.0 so the reciprocal stays finite.
                sums = nrm_pool.tile([65, 512], f32, name=f"sums{j}", tag="sums",
                                     bufs=1)
                nc.gpsimd.memset(sums[:], 1.0)
                nc.vector.tensor_copy(sums[0:1, :], acc[j][0][64:65, :])
                nc.vector.tensor_copy(sums[64:65, :], acc[j][1][64:65, :])
                rs = nrm_pool.tile([65, 512], f32, name=f"rs{j}", tag="rs", bufs=1)
                nc.vector.reciprocal(rs, sums)
                # hw partition_broadcast reads the tile's partition 0, so the
                # odd head's row must first be copied down to a base-0 tile
                rs_o = nrm_pool.tile([1, 512], f32, name=f"rso{j}", tag="rso",
                                     bufs=1)
                nc.vector.tensor_copy(rs_o, rs[64:65, :])
                for half in (0, 1):
                    po = half * 64
                    bc = nrm_pool.tile([64, 512], f32, name=f"bc{j}_{half}",
                                       tag="bc")
                    nc.gpsimd.partition_broadcast(bc, rs[0:1, :] if half == 0
                                                  else rs_o[:])
                    dst = at[j][po:po + 64, :]
                    nc.vector.tensor_mul(dst, acc[j][half][0:64, :], bc[:])
                    nc.vector.tensor_scalar_add(dst, dst,
                                                bq_sb[po:po + 64, 12 + j:13 + j])

            # ---- phase C: output projection ----
            for m in range(4):
                pp = ps2.tile([128, 1024], f32, name=f"pp{m}", tag="ps2")
                for c in range(KC):
                    nc.tensor.matmul(pp[:, 0:512],
                                     at[c][:, m * 128:(m + 1) * 128],
                                     wp[c][:, 0:512],
                                     start=(c == 0), stop=(c == KC - 1))
                for c in range(KC):
                    nc.tensor.matmul(pp[:, 512:768],
                                     at[c][:, m * 128:(m + 1) * 128],
                                     wp[c][:, 512:768],
                                     start=(c == 0), stop=(c == KC - 1))
                ot = outp.tile([128, D], f32, name=f"ot{m}", tag="ot")
                nc.vector.tensor_tensor(ot, pp[:, 0:768], bp_bc[:], ADD)
                nc.sync.dma_start(out=out[m * 128:(m + 1) * 128, :], in_=ot)

    nc.compile()
    return nc


def get_nc():
    if "nc" not in _CACHE:
        _CACHE["nc"] = _build()
    return _CACHE["nc"]


def make_in_maps(x, W_qkv, b_qkv, W_proj, b_proj):
    import ml_dtypes
    bf = ml_dtypes.bfloat16
    x = np.ascontiguousarray(np.asarray(x, dtype=np.float32).astype(bf))
    W_qkv = np.ascontiguousarray(np.asarray(W_qkv, dtype=np.float32).astype(bf))
    b_qkv = np.ascontiguousarray(np.asarray(b_qkv, dtype=np.float32))
    W_proj = np.ascontiguousarray(np.asarray(W_proj, dtype=np.float32).astype(bf))
    b_proj = np.ascontiguousarray(np.asarray(b_proj, dtype=np.float32))
    in_maps = []
    for c in range(NCORES):
        b, q0 = c // 4, (c % 4) * SQ
        xbat = np.roll(x[b], -q0, axis=0)  # own query rows first; key order is free
        in_maps.append({"xb": np.ascontiguousarray(xbat), "wqkv": W_qkv,
                        "bqkv": b_qkv, "wproj": W_proj, "bproj": b_proj})
    return in_maps


def run(in_maps, **kw):
    return bass_utils.run_bass_kernel_spmd(get_nc(), in_maps,
                                           core_ids=list(range(NCORES)), **kw)


def kernel(x, W_qkv, b_qkv, W_proj, b_proj):
    in_maps = make_in_maps(x, W_qkv, b_qkv, W_proj, b_proj)
    res = run(in_maps)
    out = np.empty((B, N, D), dtype=np.float32)
    for c in range(NCORES):
        b, q0 = c // 4, (c % 4) * SQ
        out[b, q0:q0 + SQ] = res.results[c]["out"]
    return out
